# revision 13
# baseline (speedup 1.0000x reference)
"""Trainium2 Bass kernel for nn_Calibration (retrieval_knn).

Per batch element (only view_id matters):
  1. pixel round/flip + outlier test vs binary silhouette mask
     (mask bit-packed on device, per-point row fetch via dma_gather)
  2. K=1 KNN of pixel points vs 1024 boundary points:
     PE matmul scores (argmax of 2*o.bd - |bd|^2) -> DVE max8/find_index8 on PSUM
  3. dma_gather nearest boundary point, back-project through inv_param, select.

Sharding: data-parallel over batch dim, 2 batches per core x 8 NeuronCores.
"""

import contextlib
import ctypes
import sys
import types

import numpy as np

import concourse.bacc as bacc
import concourse.mybir as mybir
from concourse import library_config
from concourse.tile import TileContext
from concourse.bass_utils import run_bass_kernel_spmd

# ---------------------------------------------------------------- constants
IMG = 224
B, V, N, M = 16, 8, 8192, 1024
NCORES = 8
BPC = B // NCORES           # batches per core = 2
TB = N // 128               # tiles per batch = 64
T = BPC * TB                # point tiles per core = 128
NI = BPC * N                # points per core = 16384
R7 = float(np.float32(1.0) / np.float32(7.0))
MAGIC = float(2.0 ** 23)

_PROG = None


# ------------------------------------------------------- NTFF hook (trace)
def _install_ntff_hook():
    name = "antenv.axon_hooks"
    if name in sys.modules:
        return
    try:
        lib = ctypes.CDLL("/opt/axon/libaxon_pjrt.so")
        if not hasattr(lib, "axon_start_nrt_profile"):
            return
        lib.axon_start_nrt_profile.argtypes = [ctypes.POINTER(ctypes.c_int64), ctypes.c_size_t]
        lib.axon_start_nrt_profile.restype = ctypes.c_int64
        lib.axon_stop_nrt_profile.argtypes = [ctypes.c_char_p]
        lib.axon_stop_nrt_profile.restype = ctypes.c_int64

        @contextlib.contextmanager
        def _hook(output_dir, device_ids):
            import jax
            jax.devices()
            if device_ids:
                ids = (ctypes.c_int64 * len(device_ids))(*device_ids)
                rc = lib.axon_start_nrt_profile(ids, len(device_ids))
            else:
                rc = lib.axon_start_nrt_profile(None, 0)
            if rc != 0:
                raise RuntimeError(f"axon_start_nrt_profile rc={rc}")
            try:
                yield
            finally:
                n = lib.axon_stop_nrt_profile(str(output_dir).encode())
                if n <= 0:
                    print(f"profile: {n} files written to {output_dir}", file=sys.stderr)

        mod = types.ModuleType(name)
        mod._HOOK = _hook
        mod.get_axon_ntff_profile_hook = lambda: mod._HOOK
        mod.set_axon_ntff_profile_hook = lambda h: setattr(mod, "_HOOK", h)
        sys.modules[name] = mod
        import antenv
        antenv.axon_hooks = mod
    except Exception:
        pass


# ------------------------------------------------------------ device program
def _exact_div7(nc, pool, out_ap, in_ap, shape, post_scale, tag):
    """out = (in/7)*post_scale with correctly-rounded in/7 (q0=x*r; t=q0*8;
    a=x-t (Sterbenz); resid=a+q0 (exact = x-7*q0); q=q0+resid*r). Verified
    bit-exact vs IEEE divide over all 2^24 mantissas. post_scale is a power
    of two so the final multiply is exact."""
    q0 = pool.tile(shape, mybir.dt.float32, tag=f"{tag}_q0")
    t1 = pool.tile(shape, mybir.dt.float32, tag=f"{tag}_t1")
    nc.vector.tensor_scalar(q0[:], in_ap, R7, scalar2=None, op0=mybir.AluOpType.mult)
    nc.vector.tensor_scalar(t1[:], q0[:], 8.0, scalar2=None, op0=mybir.AluOpType.mult)
    nc.vector.tensor_tensor(t1[:], in_ap, t1[:], op=mybir.AluOpType.subtract)
    nc.vector.tensor_tensor(t1[:], t1[:], q0[:], op=mybir.AluOpType.add)
    nc.vector.tensor_scalar(t1[:], t1[:], R7, scalar2=None, op0=mybir.AluOpType.mult)
    nc.vector.tensor_tensor(q0[:], q0[:], t1[:], op=mybir.AluOpType.add)
    nc.vector.tensor_scalar(out_ap, q0[:], post_scale, scalar2=None, op0=mybir.AluOpType.mult)


def _build_program():
    nc = bacc.Bacc("TRN2", target_bir_lowering=False, debug=False, num_devices=NCORES)
    f32, i32, i16, u32 = mybir.dt.float32, mybir.dt.int32, mybir.dt.int16, mybir.dt.uint32
    TT, TS, RED = nc.vector.tensor_tensor, nc.vector.tensor_scalar, nc.vector.tensor_reduce
    OP = mybir.AluOpType

    # inputs (per core)
    pts = nc.dram_tensor("pts", [128, 6 * T], f32, kind="ExternalInput")        # px|py|pz|pcx|pcy|pcz
    braw = nc.dram_tensor("braw", [128, BPC * 16], f32, kind="ExternalInput")   # per bt: x(8)|y(8)
    btab = nc.dram_tensor("btab", [BPC * M, 64], f32, kind="ExternalInput")     # 256B rows [bxo byo]
    maskv = nc.dram_tensor("maskv", [4 * 128, IMG], f32, kind="ExternalInput")  # 448 rows + 64 pad
    pow16 = nc.dram_tensor("pow16", [1, 16], f32, kind="ExternalInput")
    iota8 = nc.dram_tensor("iota8", [1, 16], i32, kind="ExternalInput")
    invp = nc.dram_tensor("invp", [1, BPC * 16], f32, kind="ExternalInput")
    ident = nc.dram_tensor("ident", [128, 128], f32, kind="ExternalInput")
    repmat = nc.dram_tensor("repmat", [128, 8 * 128], f32, kind="ExternalInput")

    # scratch + output
    mtab = nc.dram_tensor("mtab", [4 * 128, 64], i32)
    movscr = nc.dram_tensor("movscr", [BPC, 3, M], f32)
    outp = nc.dram_tensor("outp", [128, 3 * T], f32, kind="ExternalOutput")
    dbg = nc.dram_tensor("dbg", [128, 4 * T], f32, kind="ExternalOutput")

    with TileContext(nc) as tc:
        with (
            tc.tile_pool(name="sb", bufs=1) as pool,
            tc.tile_pool(name="wps", bufs=2, space="PSUM") as wps,
        ):
            nc.gpsimd.load_library(library_config.mlp)

            # ---------------- load inputs
            pts_t = pool.tile([128, 6 * T], f32)
            nc.gpsimd.dma_start(pts_t[:], pts[:])
            braw_t = pool.tile([128, BPC * 16], f32)
            nc.gpsimd.dma_start(braw_t[:], braw[:])
            mv = pool.tile([128, 4, IMG], f32)
            nc.gpsimd.dma_start(mv[:], maskv[:].rearrange("(q p) c -> p q c", p=128))
            powb = pool.tile([128, 14, 16], f32)
            nc.gpsimd.dma_start(powb[:], pow16[:].partition_broadcast(128).to_broadcast([128, 14, 16]))
            iotab = pool.tile([128, 16], i32)
            nc.gpsimd.dma_start(iotab[:], iota8[:].partition_broadcast(128))
            invb = pool.tile([128, BPC * 16], f32)
            nc.gpsimd.dma_start(invb[:], invp[:].partition_broadcast(128))
            id_t = pool.tile([128, 128], f32)
            nc.gpsimd.dma_start(id_t[:], ident[:])
            repm_t = pool.tile([128, 8 * 128], f32)
            nc.gpsimd.dma_start(repm_t[:], repmat[:])

            px = pts_t[:, 0 * T:1 * T]
            py = pts_t[:, 1 * T:2 * T]
            pz = pts_t[:, 2 * T:3 * T]

            # ---------------- mask bit-packing (rows on partitions)
            mprod = pool.tile([128, 4, 14, 16], f32)
            TT(mprod[:], mv[:].rearrange("p q (w j) -> p q w j", j=16),
               powb[:].unsqueeze(1).to_broadcast([128, 4, 14, 16]),
               op=OP.mult)
            hsum = pool.tile([128, 4, 14], f32)
            RED(hsum[:], mprod[:], axis=mybir.AxisListType.X, op=OP.add)
            w32 = pool.tile([128, 4, 16], i32)
            nc.vector.memset(w32[:], 0)
            nc.vector.tensor_copy(w32[:, :, 0:14], hsum[:])
            nc.gpsimd.dma_start(mtab[:].rearrange("(q p) w -> p q w", p=128)[:, :, 0:16], w32[:])

            # ---------------- bounds: bd = bounds/224 (exact), s2, moving rows
            bdiv = pool.tile([128, BPC * 16], f32)
            _exact_div7(nc, pool, bdiv[:], braw_t[:], [128, BPC * 16], float(2.0 ** -5), "bd")
            bsq = pool.tile([128, BPC * 16], f32)
            TT(bsq[:], bdiv[:], bdiv[:], op=OP.mult)
            s2t = pool.tile([128, BPC * 8], f32)
            for bt in range(BPC):
                TT(s2t[:, bt * 8:(bt + 1) * 8], bsq[:, bt * 16:bt * 16 + 8],
                   bsq[:, bt * 16 + 8:bt * 16 + 16], op=OP.add)
            for bt in range(BPC):
                nc.gpsimd.dma_start(movscr[bt, 0].rearrange("(c p) -> p c", p=128),
                                    bdiv[:, bt * 16 + 0:bt * 16 + 8])
                nc.gpsimd.dma_start(movscr[bt, 1].rearrange("(c p) -> p c", p=128),
                                    bdiv[:, bt * 16 + 8:bt * 16 + 16])
                nc.gpsimd.dma_start(movscr[bt, 2].rearrange("(c p) -> p c", p=128),
                                    s2t[:, bt * 8:(bt + 1) * 8])
            movt = pool.tile([128, BPC, M], f32)
            for g in range(4):
                for bt in range(BPC):
                    nc.gpsimd.dma_start(movt[32 * g:32 * g + 3, bt], movscr[bt])

            # ---------------- point prep
            fy = pool.tile([128, T], f32)
            TS(fy[:], py, -1.0, scalar2=float(IMG), op0=OP.mult, op1=OP.add)
            ix = pool.tile([128, T], f32)
            iy = pool.tile([128, T], f32)
            TS(ix[:], px, MAGIC, scalar2=MAGIC, op0=OP.add, op1=OP.subtract)
            TS(iy[:], fy[:], MAGIC, scalar2=MAGIC, op0=OP.add, op1=OP.subtract)
            ixc = pool.tile([128, T], f32)
            iyc = pool.tile([128, T], f32)
            TS(ixc[:], ix[:], 0.0, scalar2=223.0, op0=OP.max, op1=OP.min)
            TS(iyc[:], iy[:], 0.0, scalar2=223.0, op0=OP.max, op1=OP.min)
            inb = pool.tile([128, T], f32)
            tq = pool.tile([128, T], f32)
            TT(inb[:], ix[:], ixc[:], op=OP.is_equal)
            TT(tq[:], iy[:], iyc[:], op=OP.is_equal)
            TT(inb[:], inb[:], tq[:], op=OP.mult)
            wx = pool.tile([128, T], f32)
            wy = pool.tile([128, T], f32)
            _exact_div7(nc, pool, wx[:], ix[:], [128, T], float(2.0 ** -4), "w")
            _exact_div7(nc, pool, wy[:], iy[:], [128, T], float(2.0 ** -4), "w")

            # ---------------- weights: 4 tiles per 128x128 transpose block
            NB = T // 4
            wasm = pool.tile([128, NB, 128], f32)
            nc.vector.memset(wasm[:], 0.0)
            nc.vector.tensor_copy(
                wasm[:].rearrange("p b (i c) -> p b i c", i=4)[:, :, :, 0],
                wx[:].rearrange("p (b i) -> p b i", i=4))
            nc.vector.tensor_copy(
                wasm[:].rearrange("p b (i c) -> p b i c", i=4)[:, :, :, 1],
                wy[:].rearrange("p (b i) -> p b i", i=4))
            nc.vector.memset(wasm[:].rearrange("p b (i c) -> p b i c", i=4)[:, :, :, 2], -1.0)
            wsb = pool.tile([128, NB, 128], f32)
            for blk in range(NB):
                wp = wps.tile([128, 128], f32, tag="wtr")
                nc.tensor.transpose(wp[:], wasm[:, blk], id_t[:])
                nc.scalar.activation(wsb[:, blk], wp[:], mybir.ActivationFunctionType.Copy)

            # ---------------- score loop
            idx8 = pool.tile([128, T, 8], u32)
            m8 = pool.tile([128, T, 8], f32)
            with tc.tile_pool(name="sps", bufs=3, space="PSUM") as sps:
                for t in range(T):
                    bt = t // TB
                    blk, i = t // 4, t % 4
                    ps = sps.tile([128, M], f32, tag="score")
                    lhsT = wsb[:, blk][32 * i:32 * i + 3, :]
                    rhs = movt[32 * i:32 * i + 3, bt]
                    for h in range(2):
                        nc.tensor.matmul(ps[:, 512 * h:512 * (h + 1)], lhsT,
                                         rhs[:, 512 * h:512 * (h + 1)],
                                         start=True, stop=True,
                                         tile_position=(32 * i, 0))
                    nc.vector.max(m8[:, t], ps[:])
                    nc.vector.max_index(idx8[:, t], m8[:, t], ps[:])

            # ---------------- indices -> wrapped int16 via DRAM roundtrip
            top1 = pool.tile([128, T], f32)
            nc.vector.tensor_copy(top1[:], idx8[:, :, 0])
            TS(top1[:, TB:], top1[:, TB:], float(M), scalar2=None, op0=OP.add)
            rowi = pool.tile([128, T], f32)
            nc.vector.tensor_copy(rowi[:], iyc[:])
            TS(rowi[:, TB:], rowi[:, TB:], float(IMG), scalar2=None, op0=OP.add)
            bi32 = pool.tile([128, 2 * T], i32)
            nc.vector.tensor_copy(bi32[:, 0:T], top1[:])
            nc.vector.tensor_copy(bi32[:, T:2 * T], rowi[:])
            bi16 = pool.tile([128, 2 * T], i16)
            nc.vector.tensor_copy(bi16[:], bi32[:])
            # wrap to dma_gather idx layout ON-CHIP: for each j, a 0/1 matmul
            # pulls rows 16j+q to partitions g*16+q (all replicas g at once):
            # repmat[:, j*128 + a] = 1 iff p == 16j + (a%16)
            wrapf = pool.tile([128, 2, NI // 16], f32)
            with tc.tile_pool(name="ips", bufs=2, space="PSUM") as ips:
                for kk, vals in enumerate((top1, rowi)):
                    for j in range(8):
                        pj = ips.tile([128, T], f32, tag="wrapmm")
                        nc.tensor.matmul(pj[:], repm_t[:, j * 128:(j + 1) * 128], vals[:],
                                         start=True, stop=True)
                        nc.vector.tensor_copy(
                            wrapf[:, kk].rearrange("q (t j) -> q j t", j=8)[:, j], pj[:])
            wrap32 = pool.tile([128, 2, NI // 16], i32)
            nc.vector.tensor_copy(wrap32[:], wrapf[:])
            bm16 = pool.tile([128, 2, NI // 16], i16)
            nc.vector.tensor_copy(bm16[:], wrap32[:])
            bidx = bm16[:, 0]
            midx = bm16[:, 1]


            # ---------------- gathers
            gb = pool.tile([128, T, 64], f32)
            gmi = pool.tile([128, T, 64], i32)
            if int(__import__("os").environ.get("KNL_NO_GATHER", "0")):
                nc.vector.memset(gb[:], 0.0)
                nc.vector.memset(gmi[:], 0)
            else:
                CH = 2048
                for c in range(NI // CH):
                    sl_s = slice(c * CH // 16, (c + 1) * CH // 16)
                    sl_t = slice(c * CH // 128, (c + 1) * CH // 128)
                    nc.gpsimd.dma_gather(gb[:, sl_t].bitcast(mybir.dt.uint16),
                                         btab[:].bitcast(mybir.dt.uint16),
                                         bidx[:, sl_s], num_idxs=CH, num_idxs_reg=CH,
                                         elem_size=128, single_packet=False)
                for c in range(NI // CH):
                    sl_s = slice(c * CH // 16, (c + 1) * CH // 16)
                    sl_t = slice(c * CH // 128, (c + 1) * CH // 128)
                    nc.gpsimd.dma_gather(gmi[:, sl_t].bitcast(mybir.dt.uint16),
                                         mtab[:].bitcast(mybir.dt.uint16),
                                         midx[:, sl_s], num_idxs=CH, num_idxs_reg=CH,
                                         elem_size=128, single_packet=False)

            # ---------------- mask word select + bit extract
            ixi = pool.tile([128, T], i32)
            nc.vector.tensor_copy(ixi[:], ixc[:])
            wsel = pool.tile([128, T], i32)
            TS(wsel[:], ixi[:], 4, scalar2=None, op0=OP.logical_shift_right)
            weq = pool.tile([128, T, 16], i32)
            TT(weq[:], wsel[:].unsqueeze(2).to_broadcast([128, T, 16]),
               iotab[:].unsqueeze(1).to_broadcast([128, T, 16]), op=OP.is_equal)
            TT(weq[:], weq[:], gmi[:, :, 0:16], op=OP.mult)
            wordv = pool.tile([128, T], i32)
            with nc.allow_low_precision(reason="halfword select: values < 2^16, exact"):
                RED(wordv[:], weq[:], axis=mybir.AxisListType.X, op=OP.add)
            bitv = pool.tile([128, T], i32)
            TS(bitv[:], ixi[:], 15, scalar2=None, op0=OP.bitwise_and)
            TT(bitv[:], wordv[:], bitv[:], op=OP.logical_shift_right)
            TS(bitv[:], bitv[:], 1, scalar2=None, op0=OP.bitwise_and)
            bitf = pool.tile([128, T], f32)
            nc.vector.tensor_copy(bitf[:], bitv[:])
            inlier = pool.tile([128, T], f32)
            TT(inlier[:], inb[:], bitf[:], op=OP.mult)
            inli = pool.tile([128, T], i32)
            nc.vector.tensor_copy(inli[:], inlier[:])

            # ---------------- back-projection + select + output
            ot = pool.tile([128, 3 * T], f32)
            hx = pool.tile([128, T], f32)
            hy = pool.tile([128, T], f32)
            TT(hx[:], gb[:, :, 0], pz, op=OP.mult)
            TT(hy[:], gb[:, :, 1], pz, op=OP.mult)
            acc = pool.tile([128, T], f32)
            tmp = pool.tile([128, T], f32)
            for c in range(3):
                for bt in range(BPC):
                    sl = slice(bt * TB, (bt + 1) * TB)
                    def iv(k):
                        col = bt * 16 + 4 * k + c
                        return invb[:, col:col + 1]
                    TS(acc[:, sl], hx[:, sl], iv(0), scalar2=None, op0=OP.mult)
                    TS(tmp[:, sl], hy[:, sl], iv(1), scalar2=None, op0=OP.mult)
                    TT(acc[:, sl], acc[:, sl], tmp[:, sl], op=OP.add)
                    TS(tmp[:, sl], pz[:, sl], iv(2), scalar2=None, op0=OP.mult)
                    TT(acc[:, sl], acc[:, sl], tmp[:, sl], op=OP.add)
                    TS(acc[:, sl], acc[:, sl], iv(3), scalar2=None, op0=OP.add)
                nc.vector.select(ot[:, c * T:(c + 1) * T], inli[:],
                                 pts_t[:, (3 + c) * T:(4 + c) * T], acc[:])
            nc.gpsimd.dma_start(outp[:], ot[:])
            dbg_t = pool.tile([128, 4 * T], f32)
            nc.vector.tensor_copy(dbg_t[:, 0:T], top1[:])
            nc.vector.tensor_copy(dbg_t[:, T:2 * T], inlier[:])
            nc.vector.tensor_copy(dbg_t[:, 2 * T:3 * T], gb[:, :, 0])
            nc.vector.tensor_copy(dbg_t[:, 3 * T:4 * T], gb[:, :, 1])
            nc.gpsimd.dma_start(dbg[:], dbg_t[:])

    nc.compile()
    return nc


def _get_program():
    global _PROG
    if _PROG is None:
        _PROG = _build_program()
    return _PROG


# ------------------------------------------------------------- host wrapper
def _tileize(x):
    """(BPC, N) -> (128, T): tile t = batch t//TB, points (t%TB)*128..+128"""
    return np.ascontiguousarray(x.reshape(BPC * TB, 128).T.astype(np.float32))


def _prep_inputs(pc, mask, bounds, inv_param, proj_fine, proj_finez, view_id):
    v = int(view_id)
    pxy = proj_fine[:, v]
    pzv = proj_finez[:, v]
    mk = mask[:, v]
    bd = bounds[:, v]
    ip = inv_param[:, v]

    pow16 = (2.0 ** np.arange(16)).astype(np.float32).reshape(1, 16)
    iota8 = np.arange(16, dtype=np.int32).reshape(1, 16)
    ident = np.eye(128, dtype=np.float32)
    repmat = np.zeros((128, 8 * 128), np.float32)
    for j in range(8):
        for a in range(128):
            repmat[16 * j + (a % 16), j * 128 + a] = 1.0

    in_maps = []
    for c in range(NCORES):
        bs = [c * BPC + i for i in range(BPC)]
        pts = np.concatenate([
            _tileize(np.stack([pxy[b, :, 0] for b in bs])),
            _tileize(np.stack([pxy[b, :, 1] for b in bs])),
            _tileize(np.stack([pzv[b] for b in bs])),
            _tileize(np.stack([pc[b, :, 0] for b in bs])),
            _tileize(np.stack([pc[b, :, 1] for b in bs])),
            _tileize(np.stack([pc[b, :, 2] for b in bs])),
        ], axis=1)

        braw = np.zeros((128, BPC * 16), np.float32)
        btab = np.zeros((BPC * M, 64), np.float32)
        maskv = np.zeros((4 * 128, IMG), np.float32)
        for i, b in enumerate(bs):
            braw[:, i * 16 + 0:i * 16 + 8] = bd[b, :, 0].reshape(8, 128).T
            braw[:, i * 16 + 8:i * 16 + 16] = bd[b, :, 1].reshape(8, 128).T
            btab[i * M:(i + 1) * M, 0] = bd[b, :, 0]
            btab[i * M:(i + 1) * M, 1] = bd[b, :, 1]
            maskv[i * IMG:(i + 1) * IMG] = mk[b]
        invp = np.concatenate([ip[b].reshape(16) for b in bs]).astype(np.float32).reshape(1, -1)

        in_maps.append({
            "pts": pts, "braw": braw, "btab": btab, "maskv": maskv,
            "pow16": pow16, "iota8": iota8, "invp": invp, "ident": ident,
            "repmat": repmat,
        })
    return in_maps


def _postprocess(results):
    out = np.empty((B, N, 3), np.float32)
    for c, r in enumerate(results):
        ot = r["outp"]
        for i in range(BPC):
            b = c * BPC + i
            for ch in range(3):
                blk = ot[:, ch * T + i * TB:ch * T + (i + 1) * TB]
                out[b, :, ch] = blk.T.reshape(N)
    return out


def kernel(pc, mask, bounds, inv_param, proj_fine, proj_finez, view_id, _trace=False):
    pc = np.asarray(pc, np.float32)
    mask = np.asarray(mask, np.float32)
    bounds = np.asarray(bounds, np.float32)
    inv_param = np.asarray(inv_param, np.float32)
    proj_fine = np.asarray(proj_fine, np.float32)
    proj_finez = np.asarray(proj_finez, np.float32)

    if _trace:
        _install_ntff_hook()
    nc = _get_program()
    in_maps = _prep_inputs(pc, mask, bounds, inv_param, proj_fine, proj_finez, view_id)
    res = run_bass_kernel_spmd(nc, in_maps, list(range(NCORES)), trace=_trace)
    out = _postprocess(res.results)
    kernel.last_result = res
    return out


kernel.last_result = None


# revision 15
# speedup vs baseline: 1.5314x; 1.5314x over previous
"""Trainium2 Bass kernel for nn_Calibration (retrieval_knn).

Per batch element (only view_id matters):
  1. pixel round/flip + outlier test vs binary silhouette mask
     (mask packed to 16-bit halfwords on device, per-point ap_gather)
  2. K=1 KNN of pixel points vs 1024 boundary points:
     PE matmul scores (argmax of 2*o.bd - |bd|^2) -> DVE max8/find_index8 on PSUM
  3. ap_gather nearest boundary point, back-project through inv_param, select.

Sharding: data-parallel over batch dim, 2 batches per core x 8 NeuronCores.
"""

import contextlib
import ctypes
import sys
import types

import numpy as np

import concourse.bacc as bacc
import concourse.mybir as mybir
from concourse import library_config
from concourse.tile import TileContext
from concourse.bass_utils import run_bass_kernel_spmd

# ---------------------------------------------------------------- constants
IMG = 224
B, V, N, M = 16, 8, 8192, 1024
NCORES = 8
BPC = B // NCORES           # batches per core = 2
TB = N // 128               # tiles per batch = 64
T = BPC * TB                # point tiles per core = 128
NI = BPC * N                # points per core = 16384
NIG = NI // 8               # points per Q7 core group = 2048
NHW = 14                    # halfwords per mask row
R7 = float(np.float32(1.0) / np.float32(7.0))
MAGIC = float(2.0 ** 23)

_PROG = None


# ------------------------------------------------------- NTFF hook (trace)
def _install_ntff_hook():
    name = "antenv.axon_hooks"
    if name in sys.modules:
        return
    try:
        lib = ctypes.CDLL("/opt/axon/libaxon_pjrt.so")
        if not hasattr(lib, "axon_start_nrt_profile"):
            return
        lib.axon_start_nrt_profile.argtypes = [ctypes.POINTER(ctypes.c_int64), ctypes.c_size_t]
        lib.axon_start_nrt_profile.restype = ctypes.c_int64
        lib.axon_stop_nrt_profile.argtypes = [ctypes.c_char_p]
        lib.axon_stop_nrt_profile.restype = ctypes.c_int64

        @contextlib.contextmanager
        def _hook(output_dir, device_ids):
            import jax
            jax.devices()
            if device_ids:
                ids = (ctypes.c_int64 * len(device_ids))(*device_ids)
                rc = lib.axon_start_nrt_profile(ids, len(device_ids))
            else:
                rc = lib.axon_start_nrt_profile(None, 0)
            if rc != 0:
                raise RuntimeError(f"axon_start_nrt_profile rc={rc}")
            try:
                yield
            finally:
                n = lib.axon_stop_nrt_profile(str(output_dir).encode())
                if n <= 0:
                    print(f"profile: {n} files written to {output_dir}", file=sys.stderr)

        mod = types.ModuleType(name)
        mod._HOOK = _hook
        mod.get_axon_ntff_profile_hook = lambda: mod._HOOK
        mod.set_axon_ntff_profile_hook = lambda h: setattr(mod, "_HOOK", h)
        sys.modules[name] = mod
        import antenv
        antenv.axon_hooks = mod
    except Exception:
        pass


# ------------------------------------------------------------ device program
def _exact_div7(nc, pool, out_ap, in_ap, shape, post_scale, tag):
    """out = (in/7)*post_scale with correctly-rounded in/7 (q0=x*r; t=q0*8;
    a=x-t (Sterbenz); resid=a+q0 (exact = x-7*q0); q=q0+resid*r). Verified
    bit-exact vs IEEE divide over all 2^24 mantissas. post_scale is a power
    of two so the final multiply is exact."""
    q0 = pool.tile(shape, mybir.dt.float32, tag=f"{tag}_q0")
    t1 = pool.tile(shape, mybir.dt.float32, tag=f"{tag}_t1")
    nc.vector.tensor_scalar(q0[:], in_ap, R7, scalar2=None, op0=mybir.AluOpType.mult)
    nc.vector.tensor_scalar(t1[:], q0[:], 8.0, scalar2=None, op0=mybir.AluOpType.mult)
    nc.vector.tensor_tensor(t1[:], in_ap, t1[:], op=mybir.AluOpType.subtract)
    nc.vector.tensor_tensor(t1[:], t1[:], q0[:], op=mybir.AluOpType.add)
    nc.vector.tensor_scalar(t1[:], t1[:], R7, scalar2=None, op0=mybir.AluOpType.mult)
    nc.vector.tensor_tensor(q0[:], q0[:], t1[:], op=mybir.AluOpType.add)
    nc.vector.tensor_scalar(out_ap, q0[:], post_scale, scalar2=None, op0=mybir.AluOpType.mult)


def _build_program():
    nc = bacc.Bacc("TRN2", target_bir_lowering=False, debug=False, num_devices=NCORES)
    f32, i32, i16, u32 = mybir.dt.float32, mybir.dt.int32, mybir.dt.int16, mybir.dt.uint32
    TT, TS, RED = nc.vector.tensor_tensor, nc.vector.tensor_scalar, nc.vector.tensor_reduce
    OP = mybir.AluOpType

    # inputs (per core)
    pts = nc.dram_tensor("pts", [128, 6 * T], f32, kind="ExternalInput")        # px|py|pz|pcx|pcy|pcz
    braw = nc.dram_tensor("braw", [128, BPC * 16], f32, kind="ExternalInput")   # per bt: x(8)|y(8)
    btab = nc.dram_tensor("btab", [1, BPC * M * 2], f32, kind="ExternalInput")  # flat [bxo byo]*
    maskv = nc.dram_tensor("maskv", [4 * 128, IMG], f32, kind="ExternalInput")  # 448 rows + 64 pad
    pow16 = nc.dram_tensor("pow16", [1, 16], f32, kind="ExternalInput")
    mask16 = nc.dram_tensor("mask16", [128, 16], f32, kind="ExternalInput")     # [p,r]=(r==p%16)
    invp = nc.dram_tensor("invp", [1, BPC * 16], f32, kind="ExternalInput")
    ident = nc.dram_tensor("ident", [128, 128], f32, kind="ExternalInput")

    # scratch + output
    mscr = nc.dram_tensor("mscr", [1, 512 * NHW], f32)
    movscr = nc.dram_tensor("movscr", [BPC, 3, M], f32)
    outp = nc.dram_tensor("outp", [128, 3 * T], f32, kind="ExternalOutput")
    dbg = nc.dram_tensor("dbg", [128, 4 * T], f32, kind="ExternalOutput")

    with TileContext(nc) as tc:
        with tc.tile_pool(name="sb", bufs=1) as pool:
            nc.gpsimd.load_library(library_config.ap_gather)

            # ---------------- load inputs
            pts_t = pool.tile([128, 6 * T], f32)
            nc.gpsimd.dma_start(pts_t[:], pts[:])
            braw_t = pool.tile([128, BPC * 16], f32)
            nc.gpsimd.dma_start(braw_t[:], braw[:])
            mv = pool.tile([128, 4, IMG], f32)
            nc.gpsimd.dma_start(mv[:], maskv[:].rearrange("(q p) c -> p q c", p=128))
            powb = pool.tile([128, NHW, 16], f32)
            nc.gpsimd.dma_start(powb[:], pow16[:].partition_broadcast(128).to_broadcast([128, NHW, 16]))
            m16_t = pool.tile([128, 16], f32)
            nc.gpsimd.dma_start(m16_t[:], mask16[:])
            btab_t = pool.tile([128, BPC * M, 2], f32)
            nc.gpsimd.dma_start(btab_t[:].rearrange("p i o -> p (i o)"),
                                btab[:].partition_broadcast(128))
            invb = pool.tile([128, BPC * 16], f32)
            nc.gpsimd.dma_start(invb[:], invp[:].partition_broadcast(128))
            id_t = pool.tile([128, 128], f32)
            nc.gpsimd.dma_start(id_t[:], ident[:])

            px = pts_t[:, 0 * T:1 * T]
            py = pts_t[:, 1 * T:2 * T]
            pz = pts_t[:, 2 * T:3 * T]

            # ---------------- mask -> 16-bit halfwords (exact f32 sums < 2^16)
            mprod = pool.tile([128, 4, NHW, 16], f32)
            TT(mprod[:], mv[:].rearrange("p q (w j) -> p q w j", j=16),
               powb[:].unsqueeze(1).to_broadcast([128, 4, NHW, 16]), op=OP.mult)
            hsum = pool.tile([128, 4, NHW], f32)
            RED(hsum[:], mprod[:], axis=mybir.AxisListType.X, op=OP.add)
            # flat halfword table: index r*14 + w, r = mask row (+224*batch)
            nc.gpsimd.dma_start(
                mscr[:].rearrange("o (q p w) -> (o p) q w", p=128, w=NHW), hsum[:])
            mtab_t = pool.tile([128, 512 * NHW], f32)
            nc.gpsimd.dma_start(mtab_t[:], mscr[:].partition_broadcast(128))

            # ---------------- bounds: bd = bounds/224 (exact), s2, moving rows
            bdiv = pool.tile([128, BPC * 16], f32)
            _exact_div7(nc, pool, bdiv[:], braw_t[:], [128, BPC * 16], float(2.0 ** -5), "bd")
            bsq = pool.tile([128, BPC * 16], f32)
            TT(bsq[:], bdiv[:], bdiv[:], op=OP.mult)
            s2t = pool.tile([128, BPC * 8], f32)
            for bt in range(BPC):
                TT(s2t[:, bt * 8:(bt + 1) * 8], bsq[:, bt * 16:bt * 16 + 8],
                   bsq[:, bt * 16 + 8:bt * 16 + 16], op=OP.add)
            for bt in range(BPC):
                nc.gpsimd.dma_start(movscr[bt, 0].rearrange("(c p) -> p c", p=128),
                                    bdiv[:, bt * 16 + 0:bt * 16 + 8])
                nc.gpsimd.dma_start(movscr[bt, 1].rearrange("(c p) -> p c", p=128),
                                    bdiv[:, bt * 16 + 8:bt * 16 + 16])
                nc.gpsimd.dma_start(movscr[bt, 2].rearrange("(c p) -> p c", p=128),
                                    s2t[:, bt * 8:(bt + 1) * 8])
            movt = pool.tile([128, BPC, M], f32)
            for g in range(4):
                for bt in range(BPC):
                    nc.gpsimd.dma_start(movt[32 * g:32 * g + 3, bt], movscr[bt])

            # ---------------- point prep
            fy = pool.tile([128, T], f32)
            TS(fy[:], py, -1.0, scalar2=float(IMG), op0=OP.mult, op1=OP.add)
            ix = pool.tile([128, T], f32)
            iy = pool.tile([128, T], f32)
            TS(ix[:], px, MAGIC, scalar2=MAGIC, op0=OP.add, op1=OP.subtract)
            TS(iy[:], fy[:], MAGIC, scalar2=MAGIC, op0=OP.add, op1=OP.subtract)
            ixc = pool.tile([128, T], f32)
            iyc = pool.tile([128, T], f32)
            TS(ixc[:], ix[:], 0.0, scalar2=223.0, op0=OP.max, op1=OP.min)
            TS(iyc[:], iy[:], 0.0, scalar2=223.0, op0=OP.max, op1=OP.min)
            inb = pool.tile([128, T], f32)
            tq = pool.tile([128, T], f32)
            TT(inb[:], ix[:], ixc[:], op=OP.is_equal)
            TT(tq[:], iy[:], iyc[:], op=OP.is_equal)
            TT(inb[:], inb[:], tq[:], op=OP.mult)
            wx = pool.tile([128, T], f32)
            wy = pool.tile([128, T], f32)
            _exact_div7(nc, pool, wx[:], ix[:], [128, T], float(2.0 ** -4), "w")
            _exact_div7(nc, pool, wy[:], iy[:], [128, T], float(2.0 ** -4), "w")

            # ---------------- weights: 4 tiles per 128x128 transpose block
            NB = T // 4
            wasm = pool.tile([128, NB, 128], f32)
            nc.vector.memset(wasm[:], 0.0)
            nc.vector.tensor_copy(
                wasm[:].rearrange("p b (i c) -> p b i c", i=4)[:, :, :, 0],
                wx[:].rearrange("p (b i) -> p b i", i=4))
            nc.vector.tensor_copy(
                wasm[:].rearrange("p b (i c) -> p b i c", i=4)[:, :, :, 1],
                wy[:].rearrange("p (b i) -> p b i", i=4))
            nc.vector.memset(wasm[:].rearrange("p b (i c) -> p b i c", i=4)[:, :, :, 2], -1.0)
            wsb = pool.tile([128, NB, 128], f32)
            with tc.tile_pool(name="wps", bufs=2, space="PSUM") as wps:
                for blk in range(NB):
                    wp = wps.tile([128, 128], f32, tag="wtr")
                    nc.tensor.transpose(wp[:], wasm[:, blk], id_t[:])
                    nc.scalar.activation(wsb[:, blk], wp[:], mybir.ActivationFunctionType.Copy)

            # ---------------- score loop
            idx8 = pool.tile([128, T, 8], u32)
            m8 = pool.tile([128, T, 8], f32)
            with tc.tile_pool(name="sps", bufs=4, space="PSUM") as sps:
                for t in range(T):
                    bt = t // TB
                    blk, i = t // 4, t % 4
                    ps = sps.tile([128, M], f32, tag="score")
                    lhsT = wsb[:, blk][32 * i:32 * i + 3, :]
                    rhs = movt[32 * i:32 * i + 3, bt]
                    for h in range(2):
                        nc.tensor.matmul(ps[:, 512 * h:512 * (h + 1)], lhsT,
                                         rhs[:, 512 * h:512 * (h + 1)],
                                         start=True, stop=True,
                                         tile_position=(32 * i, 0))
                    nc.vector.max(m8[:, t], ps[:])
                    nc.vector.max_index(idx8[:, t], m8[:, t], ps[:])

            # ---------------- gather indices (natural (128,T) layout)
            top1 = pool.tile([128, T], f32)
            nc.vector.tensor_copy(top1[:], idx8[:, :, 0])
            TS(top1[:, TB:], top1[:, TB:], float(M), scalar2=None, op0=OP.add)
            rowi = pool.tile([128, T], f32)
            nc.vector.tensor_copy(rowi[:], iyc[:])
            TS(rowi[:, TB:], rowi[:, TB:], float(IMG), scalar2=None, op0=OP.add)
            ixi = pool.tile([128, T], i32)
            nc.vector.tensor_copy(ixi[:], ixc[:])
            wsel = pool.tile([128, T], i32)
            TS(wsel[:], ixi[:], 4, scalar2=None, op0=OP.logical_shift_right)
            wself = pool.tile([128, T], f32)
            nc.vector.tensor_copy(wself[:], wsel[:])
            TS(rowi[:], rowi[:], float(NHW), scalar2=None, op0=OP.mult)
            TT(rowi[:], rowi[:], wself[:], op=OP.add)
            bmi32 = pool.tile([128, 2, T], i32)
            nc.vector.tensor_copy(bmi32[:, 0], top1[:])
            nc.vector.tensor_copy(bmi32[:, 1], rowi[:])
            bmi16 = pool.tile([128, 2, T], i16)
            nc.vector.tensor_copy(bmi16[:], bmi32[:])

            # ---------------- ap_gathers (Q7 core g handles points p in [16g,16g+16))
            gb = pool.tile([128, NIG, 2], f32)
            nc.gpsimd.ap_gather(gb[:], btab_t[:], bmi16[:, 0], channels=128,
                                num_elems=BPC * M, d=2, num_idxs=NIG)
            gm = pool.tile([128, NIG], f32)
            nc.gpsimd.ap_gather(gm[:].rearrange("p (i o) -> p i o", o=1),
                                mtab_t[:].rearrange("p (e o) -> p e o", o=1),
                                bmi16[:, 1], channels=128, num_elems=512 * NHW, d=1,
                                num_idxs=NIG)
            # extract home-partition values: out[p,t] = g[p, 16t + p%16]
            m16b = m16_t[:].unsqueeze(1).to_broadcast([128, T, 16])
            gbx = pool.tile([128, T], f32)
            gby = pool.tile([128, T], f32)
            hwv = pool.tile([128, T], f32)
            ext = pool.tile([128, T, 16], f32)
            TT(ext[:], gb[:].rearrange("p (t r) o -> p t r o", r=16)[:, :, :, 0], m16b, op=OP.mult)
            RED(gbx[:], ext[:], axis=mybir.AxisListType.X, op=OP.add)
            TT(ext[:], gb[:].rearrange("p (t r) o -> p t r o", r=16)[:, :, :, 1], m16b, op=OP.mult)
            RED(gby[:], ext[:], axis=mybir.AxisListType.X, op=OP.add)
            TT(ext[:], gm[:].rearrange("p (t r) -> p t r", r=16), m16b, op=OP.mult)
            RED(hwv[:], ext[:], axis=mybir.AxisListType.X, op=OP.add)

            # ---------------- bit extract + inlier
            wordv = pool.tile([128, T], i32)
            nc.vector.tensor_copy(wordv[:], hwv[:])
            bitv = pool.tile([128, T], i32)
            TS(bitv[:], ixi[:], 15, scalar2=None, op0=OP.bitwise_and)
            TT(bitv[:], wordv[:], bitv[:], op=OP.logical_shift_right)
            TS(bitv[:], bitv[:], 1, scalar2=None, op0=OP.bitwise_and)
            bitf = pool.tile([128, T], f32)
            nc.vector.tensor_copy(bitf[:], bitv[:])
            inlier = pool.tile([128, T], f32)
            TT(inlier[:], inb[:], bitf[:], op=OP.mult)
            inli = pool.tile([128, T], i32)
            nc.vector.tensor_copy(inli[:], inlier[:])

            # ---------------- back-projection + select + output
            ot = pool.tile([128, 3 * T], f32)
            hx = pool.tile([128, T], f32)
            hy = pool.tile([128, T], f32)
            TT(hx[:], gbx[:], pz, op=OP.mult)
            TT(hy[:], gby[:], pz, op=OP.mult)
            acc = pool.tile([128, T], f32)
            tmp = pool.tile([128, T], f32)
            for c in range(3):
                for bt in range(BPC):
                    sl = slice(bt * TB, (bt + 1) * TB)

                    def iv(k):
                        col = bt * 16 + 4 * k + c
                        return invb[:, col:col + 1].to_broadcast([128, TB])
                    TT(acc[:, sl], hx[:, sl], iv(0), op=OP.mult)
                    TT(tmp[:, sl], hy[:, sl], iv(1), op=OP.mult)
                    TT(acc[:, sl], acc[:, sl], tmp[:, sl], op=OP.add)
                    TT(tmp[:, sl], pz[:, sl], iv(2), op=OP.mult)
                    TT(acc[:, sl], acc[:, sl], tmp[:, sl], op=OP.add)
                    TT(acc[:, sl], acc[:, sl], iv(3), op=OP.add)
                nc.vector.select(ot[:, c * T:(c + 1) * T], inli[:],
                                 pts_t[:, (3 + c) * T:(4 + c) * T], acc[:])
            nc.gpsimd.dma_start(outp[:], ot[:])
            dbg_t = pool.tile([128, 4 * T], f32)
            nc.vector.tensor_copy(dbg_t[:, 0:T], top1[:])
            nc.vector.tensor_copy(dbg_t[:, T:2 * T], inlier[:])
            nc.vector.tensor_copy(dbg_t[:, 2 * T:3 * T], gbx[:])
            nc.vector.tensor_copy(dbg_t[:, 3 * T:4 * T], gby[:])
            nc.gpsimd.dma_start(dbg[:], dbg_t[:])

    nc.compile()
    return nc


def _get_program():
    global _PROG
    if _PROG is None:
        _PROG = _build_program()
    return _PROG


# ------------------------------------------------------------- host wrapper
def _tileize(x):
    """(BPC, N) -> (128, T): tile t = batch t//TB, points (t%TB)*128..+128"""
    return np.ascontiguousarray(x.reshape(BPC * TB, 128).T.astype(np.float32))


def _prep_inputs(pc, mask, bounds, inv_param, proj_fine, proj_finez, view_id):
    v = int(view_id)
    pxy = proj_fine[:, v]
    pzv = proj_finez[:, v]
    mk = mask[:, v]
    bd = bounds[:, v]
    ip = inv_param[:, v]

    pow16 = (2.0 ** np.arange(16)).astype(np.float32).reshape(1, 16)
    mask16 = np.zeros((128, 16), np.float32)
    for p in range(128):
        mask16[p, p % 16] = 1.0
    ident = np.eye(128, dtype=np.float32)

    in_maps = []
    for c in range(NCORES):
        bs = [c * BPC + i for i in range(BPC)]
        pts = np.concatenate([
            _tileize(np.stack([pxy[b, :, 0] for b in bs])),
            _tileize(np.stack([pxy[b, :, 1] for b in bs])),
            _tileize(np.stack([pzv[b] for b in bs])),
            _tileize(np.stack([pc[b, :, 0] for b in bs])),
            _tileize(np.stack([pc[b, :, 1] for b in bs])),
            _tileize(np.stack([pc[b, :, 2] for b in bs])),
        ], axis=1)

        braw = np.zeros((128, BPC * 16), np.float32)
        btab = np.zeros((BPC * M, 2), np.float32)
        maskv = np.zeros((4 * 128, IMG), np.float32)
        for i, b in enumerate(bs):
            braw[:, i * 16 + 0:i * 16 + 8] = bd[b, :, 0].reshape(8, 128).T
            braw[:, i * 16 + 8:i * 16 + 16] = bd[b, :, 1].reshape(8, 128).T
            btab[i * M:(i + 1) * M, 0] = bd[b, :, 0]
            btab[i * M:(i + 1) * M, 1] = bd[b, :, 1]
            maskv[i * IMG:(i + 1) * IMG] = mk[b]
        btab = np.ascontiguousarray(btab.reshape(1, -1))
        invp = np.concatenate([ip[b].reshape(16) for b in bs]).astype(np.float32).reshape(1, -1)

        in_maps.append({
            "pts": pts, "braw": braw, "btab": btab, "maskv": maskv,
            "pow16": pow16, "mask16": mask16, "invp": invp, "ident": ident,
        })
    return in_maps


def _postprocess(results):
    out = np.empty((B, N, 3), np.float32)
    for c, r in enumerate(results):
        ot = r["outp"]
        for i in range(BPC):
            b = c * BPC + i
            for ch in range(3):
                blk = ot[:, ch * T + i * TB:ch * T + (i + 1) * TB]
                out[b, :, ch] = blk.T.reshape(N)
    return out


def kernel(pc, mask, bounds, inv_param, proj_fine, proj_finez, view_id, _trace=False):
    pc = np.asarray(pc, np.float32)
    mask = np.asarray(mask, np.float32)
    bounds = np.asarray(bounds, np.float32)
    inv_param = np.asarray(inv_param, np.float32)
    proj_fine = np.asarray(proj_fine, np.float32)
    proj_finez = np.asarray(proj_finez, np.float32)

    if _trace:
        _install_ntff_hook()
    nc = _get_program()
    in_maps = _prep_inputs(pc, mask, bounds, inv_param, proj_fine, proj_finez, view_id)
    res = run_bass_kernel_spmd(nc, in_maps, list(range(NCORES)), trace=_trace)
    out = _postprocess(res.results)
    kernel.last_result = res
    return out


kernel.last_result = None
